# revision 33
# baseline (speedup 1.0000x reference)
"""Trainium2 Bass kernel for topk_masking (L2Prompt-style prompted aggregation).

Computes, for ppg [B,1,D], keys/prompt [P,D], k:
  cos   = cosine_similarity(ppg[:,0,:], keys)          [B,P]
  score = 1 - cos; top-k smallest scores per row
  out   = (ppg + 0.5 * sum of the k selected prompt rows,
           sum of selected scores, entropy of softmax(score) summed over B,P)

Sharding: data-parallel over batch B across 8 NeuronCores; keys/prompt
replicated. Scalar partials reduced on host.

Self-contained: hardcodes shapes B=8192, P=1024, D=2048, 8 cores.
"""

import os
import sys

import numpy as np

for _p in ("/opt/trn_rl_repo",):
    if _p not in sys.path and os.path.isdir(_p):
        sys.path.insert(0, _p)

import concourse.bass as bass
import concourse.mybir as mybir
from concourse import bacc as bacc_mod
from concourse import bass_utils
from concourse.masks import make_identity
from concourse.tile import TileContext

B, P, D = 8192, 1024, 2048
NCORES = 8
BL = B // NCORES          # rows per core = 1024
BT = BL // 128            # b-tiles per core = 8
PT = P // 128             # p-tiles = 8
DT = D // 128             # d-blocks = 16

F32 = mybir.dt.float32
F32R = mybir.dt.float32r
BF16 = mybir.dt.bfloat16


COS_F32R = os.environ.get("KERNEL_COS_F32R", "0") == "1"
AGG_F32R = os.environ.get("KERNEL_AGG_F32R", "1") == "1"


def build_kernel(k=5, cos_f32r=False, agg_f32r=True, stage=99):
    """Build the per-core Bass program (identical on all 8 cores).

    stage (debug): 1=load/store only, 2=+norms, 3=+transposes, 4=+cos mm,
    5=+stats ops, 6=+mask/maskT, 99=full.
    """
    assert 1 <= k <= 8
    nc = bacc_mod.Bacc(trn_type="TRN2")

    ppg_d = nc.dram_tensor("ppg", [BL, D], F32, kind="ExternalInput")
    keys_d = nc.dram_tensor("keys", [P, D], F32, kind="ExternalInput")
    prompt_d = nc.dram_tensor("prompt", [P, D], F32, kind="ExternalInput")
    out_d = nc.dram_tensor("out", [BL, D], F32, kind="ExternalOutput")
    # stats[:, j]   = sum of top-k cos for b-tile j's rows
    # stats[:, 8+j] = per-row entropy for b-tile j's rows
    stats_d = nc.dram_tensor("stats", [128, 16], F32, kind="ExternalOutput")

    with TileContext(nc) as tc:
        with (
            tc.tile_pool(name="singles", bufs=1) as singles,
            tc.tile_pool(name="rows", bufs=3) as rows,
            tc.tile_pool(name="big", bufs=2) as big,
            tc.tile_pool(name="mid", bufs=2) as mid,
            tc.tile_pool(name="small", bufs=3) as small,
            tc.tile_pool(name="scratch", bufs=1) as scratch,
            tc.tile_pool(name="psum_mm", bufs=3, space="PSUM") as psum_mm,
            tc.tile_pool(name="psum_tr", bufs=2, space="PSUM") as psum_tr,
        ):
            if stage >= 3:
                identity = singles.tile([128, 128], F32)
                make_identity(nc, identity)

            # prompt resident, natural [P, D] layout as [128, PT, D].
            # Typed f32r when the agg matmul runs in f32r mode (same 32-bit
            # storage; PE rounds mantissa on read).
            if stage >= 99:
                prompt_sb = singles.tile([128, PT, D], F32R if agg_f32r else F32)
                prompt_src = prompt_d[:, :].rearrange("(n p) d -> p n d", p=128)
                if agg_f32r:
                    prompt_src = prompt_src.bitcast(F32R)
                nc.sync.dma_start(prompt_sb, prompt_src)

            # keysT resident: [128 (d within block), DT, P] = keys^T / kn
            if stage >= 3:
                keysT = singles.tile([128, DT, P], F32R if cos_f32r else F32)

            if stage >= 5:
                stats_sb = singles.tile([128, 16], F32)

            # ---------------- phase A: keys -> normalized keysT ----------
            # process p-tiles in pairs so transpose PSUM->SBUF copies are 256 wide
            for chunk in range(PT // 2 if stage >= 3 else 0):
                khat = []
                for j in range(2):
                    pt = chunk * 2 + j
                    ktile = rows.tile([128, D], F32, tag="rowtile")
                    nc.sync.dma_start(ktile, keys_d[pt * 128 : (pt + 1) * 128, :])
                    sq = scratch.tile([128, D], BF16, tag="sq")
                    kn2 = small.tile([128, 1], F32, tag="norm2")
                    nc.scalar.activation(
                        out=sq, in_=ktile,
                        func=mybir.ActivationFunctionType.Square,
                        accum_out=kn2,
                    )
                    kn = small.tile([128, 1], F32, tag="norm")
                    nc.scalar.sqrt(kn, kn2)
                    knr = small.tile([128, 1], F32, tag="nrecip")
                    nc.vector.reciprocal(knr, kn)
                    khat_t = rows.tile([128, D], F32, tag="rowtile")
                    nc.scalar.mul(khat_t, ktile, knr)
                    khat.append(khat_t)
                for dblk in range(DT):
                    pst = psum_tr.tile([128, 256], F32, tag="tr")
                    for j in range(2):
                        nc.tensor.transpose(
                            pst[:, j * 128 : (j + 1) * 128],
                            khat[j][:, dblk * 128 : (dblk + 1) * 128],
                            identity,
                        )
                    nc.vector.tensor_copy(
                        keysT[:, dblk, chunk * 256 : (chunk + 1) * 256], pst
                    )

            # ---------------- phase B: per b-tile ------------------------
            for bt in range(BT):
                ppg_t = rows.tile([128, D], F32, tag="rowtile")
                nc.sync.dma_start(ppg_t, ppg_d[bt * 128 : (bt + 1) * 128, :])

                if stage < 2:
                    nc.sync.dma_start(out_d[bt * 128 : (bt + 1) * 128, :], ppg_t)
                    continue

                # row norms of q
                sq = scratch.tile([128, D], BF16, tag="sq")
                qn2 = small.tile([128, 1], F32, tag="norm2")
                nc.scalar.activation(
                    out=sq, in_=ppg_t,
                    func=mybir.ActivationFunctionType.Square,
                    accum_out=qn2,
                )
                qn = small.tile([128, 1], F32, tag="norm")
                nc.scalar.sqrt(qn, qn2)
                qnr = small.tile([128, 1], F32, tag="nrecip")
                nc.vector.reciprocal(qnr, qn)

                if stage < 3:
                    nc.sync.dma_start(out_d[bt * 128 : (bt + 1) * 128, :], ppg_t)
                    continue

                # qT: [128 (d in block), DT, 128 (b)] raw (unnormalized) transpose
                qT = big.tile([128, DT, 128], F32R if cos_f32r else F32, tag="qT")
                for dgrp in range(DT // 4):
                    pst = psum_tr.tile([128, 512], F32, tag="tr")
                    for j in range(4):
                        dblk = dgrp * 4 + j
                        nc.tensor.transpose(
                            pst[:, j * 128 : (j + 1) * 128],
                            ppg_t[:, dblk * 128 : (dblk + 1) * 128],
                            identity,
                        )
                    nc.vector.tensor_copy(qT[:, dgrp * 4 : (dgrp + 1) * 4, :], pst)

                if stage < 4:
                    nc.sync.dma_start(out_d[bt * 128 : (bt + 1) * 128, :], ppg_t)
                    continue

                # cos = (qT^T @ keysT) / qn   -> [128 b, P]
                cos_sb = mid.tile([128, P], F32, tag="cos")
                for half in range(2):
                    psc = psum_mm.tile([128, 512], F32, tag="mm")
                    for kt in range(DT):
                        nc.tensor.matmul(
                            psc,
                            qT[:, kt, :],
                            keysT[:, kt, half * 512 : (half + 1) * 512],
                            start=(kt == 0),
                            stop=(kt == DT - 1),
                        )
                    # PSUM -> SBUF fused with row scale 1/qn
                    nc.scalar.mul(
                        cos_sb[:, half * 512 : (half + 1) * 512], psc, qnr
                    )

                if stage < 5:
                    nc.sync.dma_start(out_d[bt * 128 : (bt + 1) * 128, :], ppg_t)
                    continue

                # top-8 cos per row (descending)
                v8 = small.tile([128, 8], F32, tag="v8")
                nc.vector.max(out=v8, in_=cos_sb)
                # sum of top-k cos -> stats col bt
                nc.vector.tensor_reduce(
                    stats_sb[:, bt : bt + 1], v8[:, 0:k],
                    axis=mybir.AxisListType.X, op=mybir.AluOpType.add,
                )

                if stage >= 52:
                    # softmax-entropy over score=1-cos == softmax over z=-cos
                    # p_i = exp(-cos_i) (no max-shift needed, |cos|<=1)
                    p_t = mid.tile([128, P], F32, tag="p")
                    s_col = small.tile([128, 1], F32, tag="scol")
                    nc.scalar.activation(
                        out=p_t, in_=cos_sb,
                        func=mybir.ActivationFunctionType.Exp,
                        scale=-1.0, accum_out=s_col,
                    )
                if stage >= 53:
                    # u2 = sum (p * cos)  (note u = sum z e^z = -u2)
                    tt_out = scratch.tile([128, P], F32, tag="ttout")
                    u_col = small.tile([128, 1], F32, tag="ucol")
                    nc.vector.scalar_tensor_tensor(
                        out=tt_out, in0=p_t, scalar=1.0, in1=cos_sb,
                        op0=mybir.AluOpType.mult, op1=mybir.AluOpType.mult,
                        accum_out=u_col,
                    )
                if stage >= 54:
                    # H_row = log s - u/s = log s + u2/s
                    logs = small.tile([128, 1], F32, tag="logs")
                    nc.scalar.activation(
                        out=logs, in_=s_col, func=mybir.ActivationFunctionType.Ln
                    )
                    sinv = small.tile([128, 1], F32, tag="sinv")
                    nc.vector.reciprocal(sinv, s_col)
                    usv = small.tile([128, 1], F32, tag="usv")
                    nc.vector.tensor_mul(usv, u_col, sinv)
                    nc.vector.tensor_add(stats_sb[:, 8 + bt : 9 + bt], logs, usv)
                else:
                    nc.vector.memset(stats_sb[:, 8 + bt : 9 + bt], 0.0)


                if stage < 6:
                    nc.sync.dma_start(out_d[bt * 128 : (bt + 1) * 128, :], ppg_t)
                    continue

                # mask = cos >= kth largest (exactly k ones per row), in-place
                # over cos_sb — every reader of cos is sequenced before this.
                thr = v8[:, k - 1 : k]
                mask_t = cos_sb
                nc.vector.tensor_scalar(
                    mask_t, cos_sb, thr, None, op0=mybir.AluOpType.is_ge
                )

                # maskT: [128 (p in block), PT, 128 (b)]
                maskT = mid.tile(
                    [128, PT, 128], F32R if agg_f32r else F32, tag="maskT"
                )
                for pgrp in range(PT // 4):
                    pst = psum_tr.tile([128, 512], F32, tag="tr")
                    for j in range(4):
                        pblk = pgrp * 4 + j
                        nc.tensor.transpose(
                            pst[:, j * 128 : (j + 1) * 128],
                            mask_t[:, pblk * 128 : (pblk + 1) * 128],
                            identity,
                        )
                    nc.vector.tensor_copy(
                        maskT[:, pgrp * 4 : (pgrp + 1) * 4, :], pst
                    )

                if stage < 99:
                    nc.sync.dma_start(out_d[bt * 128 : (bt + 1) * 128, :], ppg_t)
                    continue

                # agg = maskT^T @ prompt; prompted = ppg + 0.5*agg
                # (written in place into the ppg tile: out AP == in1 AP,
                #  elementwise within one DVE instruction)
                for nd in range(D // 512):
                    psa = psum_mm.tile([128, 512], F32, tag="mm")
                    for pblk in range(PT):
                        nc.tensor.matmul(
                            psa,
                            maskT[:, pblk, :],
                            prompt_sb[:, pblk, nd * 512 : (nd + 1) * 512],
                            start=(pblk == 0),
                            stop=(pblk == PT - 1),
                        )
                    nc.vector.scalar_tensor_tensor(
                        out=ppg_t[:, nd * 512 : (nd + 1) * 512],
                        in0=psa, scalar=0.5,
                        in1=ppg_t[:, nd * 512 : (nd + 1) * 512],
                        op0=mybir.AluOpType.mult, op1=mybir.AluOpType.add,
                    )
                nc.sync.dma_start(out_d[bt * 128 : (bt + 1) * 128, :], ppg_t)

            if stage >= 5:
                nc.sync.dma_start(stats_d[:, :], stats_sb)

    nc.finalize()
    return nc


_CACHE = {}


STAGE = int(os.environ.get("KERNEL_STAGE", "99"))


def _get_nc(k):
    key = (int(k), COS_F32R, AGG_F32R, STAGE)
    if key not in _CACHE:
        _CACHE[key] = build_kernel(
            k=key[0], cos_f32r=key[1], agg_f32r=key[2], stage=key[3]
        )
    return _CACHE[key]


_LAST_RESULTS = {}


def run_on_cores(inputs, k, trace=False):
    """Run the SPMD kernel on 8 cores; returns (list of out_maps, BassKernelResults)."""
    nc = _get_nc(k)
    ppg = np.ascontiguousarray(
        np.asarray(inputs["ppg"], dtype=np.float32).reshape(B, D)
    )
    keys = np.ascontiguousarray(np.asarray(inputs["keys"], dtype=np.float32))
    prompt = np.ascontiguousarray(np.asarray(inputs["prompt"], dtype=np.float32))
    in_maps = [
        {
            "ppg": ppg[c * BL : (c + 1) * BL],
            "keys": keys,
            "prompt": prompt,
        }
        for c in range(NCORES)
    ]
    res = bass_utils.run_bass_kernel_spmd(
        nc, in_maps, core_ids=list(range(NCORES)), trace=trace
    )
    _LAST_RESULTS["res"] = res
    return res


def kernel(ppg, keys, prompt, k):
    k = int(k)
    inputs = {"ppg": ppg, "keys": keys, "prompt": prompt}
    res = run_on_cores(inputs, k, trace=False)
    outs = res.results
    prompted = np.concatenate([m["out"] for m in outs], axis=0).reshape(B, 1, D)
    top5 = np.sum(
        np.stack([m["stats"][:, :8] for m in outs]).astype(np.float64)
    )
    ent = np.sum(
        np.stack([m["stats"][:, 8:] for m in outs]).astype(np.float64)
    )
    score_sum = np.float32(k * B - top5)
    entropy = np.float32(ent)
    return prompted, score_sum, entropy


# revision 35
# speedup vs baseline: 1.2096x; 1.2096x over previous
"""Trainium2 Bass kernel for topk_masking (L2Prompt-style prompted aggregation).

Computes, for ppg [B,1,D], keys/prompt [P,D], k:
  cos   = cosine_similarity(ppg[:,0,:], keys)          [B,P]
  score = 1 - cos; top-k smallest scores per row
  out   = (ppg + 0.5 * sum of the k selected prompt rows,
           sum of selected scores, entropy of softmax(score) summed over B,P)

Sharding: data-parallel over batch B across 8 NeuronCores; keys/prompt
replicated. Scalar partials reduced on host.

Per-core pipeline (BL=1024 rows, 8 b-tiles of 128):
  phase A   keys -> keysT = keys^T/||keys|| via PE transposes (DVE row-scale)
  s1a(bt)   load ppg tile, row norms, PE-transpose raw q -> qT
  s1b(bt)   cos matmul (contract D, f32), PSUM->SBUF scaled by 1/||q|| (DVE)
  stage2(bt) max8 top-k, exp/entropy partials, threshold mask (in-place),
            PE-transpose mask -> maskT, agg matmul maskT^T @ prompt (f32r),
            prompted = ppg + 0.5*agg (in-place), DMA out
  s1a/s1b run 2/1 b-tiles ahead so PE stays busy during the stats chain.

Self-contained: hardcodes shapes B=8192, P=1024, D=2048, 8 cores.
"""

import os
import sys

import numpy as np

for _p in ("/opt/trn_rl_repo",):
    if _p not in sys.path and os.path.isdir(_p):
        sys.path.insert(0, _p)

import concourse.mybir as mybir
from concourse import bacc as bacc_mod
from concourse import bass_utils
from concourse.masks import make_identity
from concourse.tile import TileContext

B, P, D = 8192, 1024, 2048
NCORES = 8
BL = B // NCORES          # rows per core = 1024
BT = BL // 128            # b-tiles per core = 8
PT = P // 128             # p-tiles = 8
DT = D // 128             # d-blocks = 16

F32 = mybir.dt.float32
F32R = mybir.dt.float32r
BF16 = mybir.dt.bfloat16

COS_F32R = os.environ.get("KERNEL_COS_F32R", "0") == "1"
AGG_F32R = os.environ.get("KERNEL_AGG_F32R", "1") == "1"


def build_kernel(k=5, cos_f32r=False, agg_f32r=True):
    """Build the per-core Bass program (identical on all 8 cores)."""
    assert 1 <= k <= 8
    nc = bacc_mod.Bacc(trn_type="TRN2")

    ppg_d = nc.dram_tensor("ppg", [BL, D], F32, kind="ExternalInput")
    keys_d = nc.dram_tensor("keys", [P, D], F32, kind="ExternalInput")
    prompt_d = nc.dram_tensor("prompt", [P, D], F32, kind="ExternalInput")
    out_d = nc.dram_tensor("out", [BL, D], F32, kind="ExternalOutput")
    # stats[:, j]   = sum of top-k cos for b-tile j's rows
    # stats[:, 8+j] = per-row entropy for b-tile j's rows
    stats_d = nc.dram_tensor("stats", [128, 16], F32, kind="ExternalOutput")

    cos_dt = F32R if cos_f32r else F32
    agg_dt = F32R if agg_f32r else F32

    with TileContext(nc) as tc:
        with (
            tc.tile_pool(name="singles", bufs=1) as singles,
            tc.tile_pool(name="rows", bufs=4) as rows,
            tc.tile_pool(name="big", bufs=2) as big,
            tc.tile_pool(name="mid", bufs=2) as mid,
            tc.tile_pool(name="small", bufs=3) as small,
            tc.tile_pool(name="scratch", bufs=1) as scratch,
            tc.tile_pool(name="psum_mm", bufs=4, space="PSUM") as psum_mm,
            tc.tile_pool(name="psum_tr", bufs=2, space="PSUM") as psum_tr,
        ):
            identity = singles.tile([128, 128], F32)
            make_identity(nc, identity)

            # prompt resident, natural [P, D] layout as [128, PT, D]
            prompt_sb = singles.tile([128, PT, D], agg_dt)
            prompt_src = prompt_d[:, :].rearrange("(n p) d -> p n d", p=128)
            if agg_f32r:
                prompt_src = prompt_src.bitcast(F32R)
            nc.sync.dma_start(prompt_sb, prompt_src)

            # keysT resident: [128 (d within block), DT, P] = keys^T / kn
            keysT = singles.tile([128, DT, P], cos_dt)

            stats_sb = singles.tile([128, 16], F32)
            s_all = singles.tile([128, BT], F32)   # sum exp(-cos) per row
            u_all = singles.tile([128, BT], F32)   # sum cos*exp(-cos) per row

            # per-bt state carried between pipeline stages
            ppg_tiles = [None] * BT
            qnr_tiles = [None] * BT
            qT_tiles = [None] * BT
            cos_tiles = [None] * BT

            def phase_a_chunk(chunk):
                """Two keys p-tiles -> normalized keysT columns."""
                khat = []
                for j in range(2):
                    pt = chunk * 2 + j
                    ktile = rows.tile([128, D], F32, tag="rowtile")
                    nc.sync.dma_start(ktile, keys_d[pt * 128 : (pt + 1) * 128, :])
                    sq = scratch.tile([128, D], BF16, tag="sq")
                    kn2 = small.tile([128, 1], F32, tag="norm2")
                    nc.scalar.activation(
                        out=sq, in_=ktile,
                        func=mybir.ActivationFunctionType.Square,
                        accum_out=kn2,
                    )
                    kn = small.tile([128, 1], F32, tag="norm")
                    nc.scalar.sqrt(kn, kn2)
                    knr = small.tile([128, 1], F32, tag="nrecip")
                    nc.vector.reciprocal(knr, kn)
                    khat_t = rows.tile([128, D], F32, tag="rowtile")
                    nc.vector.tensor_scalar(
                        khat_t, ktile, knr, None, op0=mybir.AluOpType.mult
                    )
                    khat.append(khat_t)
                for dblk in range(DT):
                    pst = psum_tr.tile([128, 256], F32, tag="tr")
                    for j in range(2):
                        nc.tensor.transpose(
                            pst[:, j * 128 : (j + 1) * 128],
                            khat[j][:, dblk * 128 : (dblk + 1) * 128],
                            identity,
                        )
                    nc.vector.tensor_copy(
                        keysT[:, dblk, chunk * 256 : (chunk + 1) * 256], pst
                    )

            def s1a(bt):
                """Load ppg tile, row norms, transpose raw q -> qT."""
                ppg_t = rows.tile([128, D], F32, tag="rowtile")
                nc.sync.dma_start(ppg_t, ppg_d[bt * 128 : (bt + 1) * 128, :])
                sq = scratch.tile([128, D], BF16, tag="sq")
                qn2 = small.tile([128, 1], F32, tag="norm2")
                nc.scalar.activation(
                    out=sq, in_=ppg_t,
                    func=mybir.ActivationFunctionType.Square,
                    accum_out=qn2,
                )
                qn = small.tile([128, 1], F32, tag="norm")
                nc.scalar.sqrt(qn, qn2)
                qnr = small.tile([128, 1], F32, tag="nrecip")
                nc.vector.reciprocal(qnr, qn)

                qT = big.tile([128, DT, 128], cos_dt, tag="qT")
                for dgrp in range(DT // 4):
                    pst = psum_tr.tile([128, 512], F32, tag="tr")
                    for j in range(4):
                        dblk = dgrp * 4 + j
                        nc.tensor.transpose(
                            pst[:, j * 128 : (j + 1) * 128],
                            ppg_t[:, dblk * 128 : (dblk + 1) * 128],
                            identity,
                        )
                    nc.vector.tensor_copy(qT[:, dgrp * 4 : (dgrp + 1) * 4, :], pst)
                ppg_tiles[bt], qnr_tiles[bt], qT_tiles[bt] = ppg_t, qnr, qT

            def s1b(bt):
                """cos matmul + PSUM->SBUF row-scaled copy (DVE)."""
                qT, qnr = qT_tiles[bt], qnr_tiles[bt]
                cos_sb = mid.tile([128, P], F32, tag="cos")
                for half in range(2):
                    psc = psum_mm.tile([128, 512], F32, tag="mm")
                    for kt in range(DT):
                        nc.tensor.matmul(
                            psc,
                            qT[:, kt, :],
                            keysT[:, kt, half * 512 : (half + 1) * 512],
                            start=(kt == 0),
                            stop=(kt == DT - 1),
                        )
                    nc.vector.tensor_scalar(
                        cos_sb[:, half * 512 : (half + 1) * 512],
                        psc, qnr, None, op0=mybir.AluOpType.mult,
                    )
                cos_tiles[bt] = cos_sb

            def stage2(bt):
                """top-k stats, entropy partials, mask, agg, output."""
                cos_sb, ppg_t = cos_tiles[bt], ppg_tiles[bt]
                v8 = small.tile([128, 8], F32, tag="v8")
                nc.vector.max(out=v8, in_=cos_sb)
                nc.vector.tensor_reduce(
                    stats_sb[:, bt : bt + 1], v8[:, 0:k],
                    axis=mybir.AxisListType.X, op=mybir.AluOpType.add,
                )

                # p_i = exp(-cos_i); s = sum_i p_i   (|cos|<=1: no max-shift)
                p_t = mid.tile([128, P], F32, tag="p")
                nc.scalar.activation(
                    out=p_t, in_=cos_sb,
                    func=mybir.ActivationFunctionType.Exp,
                    scale=-1.0, accum_out=s_all[:, bt : bt + 1],
                )
                # u2 = sum_i p_i * cos_i  (H_row = log s + u2/s)
                # (shares the 4KB/partition "sq" scratch slot; value unused)
                tt_out = scratch.tile([128, P], F32, tag="sq")
                nc.vector.scalar_tensor_tensor(
                    out=tt_out, in0=p_t, scalar=1.0, in1=cos_sb,
                    op0=mybir.AluOpType.mult, op1=mybir.AluOpType.mult,
                    accum_out=u_all[:, bt : bt + 1],
                )

                # mask = cos >= kth largest, in place over cos_sb (all other
                # readers of cos are sequenced before this write)
                thr = v8[:, k - 1 : k]
                nc.vector.tensor_scalar(
                    cos_sb, cos_sb, thr, None, op0=mybir.AluOpType.is_ge
                )

                # maskT: [128 (p in block), PT, 128 (b)]
                maskT = mid.tile([128, PT, 128], agg_dt, tag="maskT")
                for pgrp in range(PT // 4):
                    pst = psum_tr.tile([128, 512], F32, tag="tr")
                    for j in range(4):
                        pblk = pgrp * 4 + j
                        nc.tensor.transpose(
                            pst[:, j * 128 : (j + 1) * 128],
                            cos_sb[:, pblk * 128 : (pblk + 1) * 128],
                            identity,
                        )
                    nc.vector.tensor_copy(
                        maskT[:, pgrp * 4 : (pgrp + 1) * 4, :], pst
                    )

                # agg = maskT^T @ prompt; prompted = ppg + 0.5*agg (in place)
                for nd in range(D // 512):
                    psa = psum_mm.tile([128, 512], F32, tag="mm")
                    for pblk in range(PT):
                        nc.tensor.matmul(
                            psa,
                            maskT[:, pblk, :],
                            prompt_sb[:, pblk, nd * 512 : (nd + 1) * 512],
                            start=(pblk == 0),
                            stop=(pblk == PT - 1),
                        )
                    nc.vector.scalar_tensor_tensor(
                        out=ppg_t[:, nd * 512 : (nd + 1) * 512],
                        in0=psa, scalar=0.5,
                        in1=ppg_t[:, nd * 512 : (nd + 1) * 512],
                        op0=mybir.AluOpType.mult, op1=mybir.AluOpType.add,
                    )
                nc.sync.dma_start(out_d[bt * 128 : (bt + 1) * 128, :], ppg_t)

            # ---- emission: phase A interleaved with early s1a, then the
            # ---- software-pipelined b-tile loop (s1a 2 ahead, s1b 1 ahead)
            for chunk in range(PT // 2):
                phase_a_chunk(chunk)
                if chunk >= 2:
                    s1a(chunk - 2)  # bt 0,1 early
            s1b(0)
            for bt in range(BT):
                if bt + 2 < BT:
                    s1a(bt + 2)
                if bt + 1 < BT:
                    s1b(bt + 1)
                stage2(bt)

            # batched entropy epilogue: H_row = log s + u2/s
            sinv = small.tile([128, BT], F32, tag="sinv8")
            nc.vector.reciprocal(sinv, s_all)
            logs = small.tile([128, BT], F32, tag="logs8")
            nc.scalar.activation(
                out=logs, in_=s_all, func=mybir.ActivationFunctionType.Ln
            )
            usv = small.tile([128, BT], F32, tag="usv8")
            nc.vector.tensor_mul(usv, u_all, sinv)
            nc.vector.tensor_add(stats_sb[:, 8:16], logs, usv)

            nc.sync.dma_start(stats_d[:, :], stats_sb)

    nc.finalize()
    return nc


_CACHE = {}


def _get_nc(k):
    key = (int(k), COS_F32R, AGG_F32R)
    if key not in _CACHE:
        _CACHE[key] = build_kernel(k=key[0], cos_f32r=key[1], agg_f32r=key[2])
    return _CACHE[key]


_LAST_RESULTS = {}


def run_on_cores(inputs, k, trace=False):
    """Run the SPMD kernel on 8 cores; returns BassKernelResults."""
    nc = _get_nc(k)
    ppg = np.ascontiguousarray(
        np.asarray(inputs["ppg"], dtype=np.float32).reshape(B, D)
    )
    keys = np.ascontiguousarray(np.asarray(inputs["keys"], dtype=np.float32))
    prompt = np.ascontiguousarray(np.asarray(inputs["prompt"], dtype=np.float32))
    in_maps = [
        {
            "ppg": ppg[c * BL : (c + 1) * BL],
            "keys": keys,
            "prompt": prompt,
        }
        for c in range(NCORES)
    ]
    res = bass_utils.run_bass_kernel_spmd(
        nc, in_maps, core_ids=list(range(NCORES)), trace=trace
    )
    _LAST_RESULTS["res"] = res
    return res


def kernel(ppg, keys, prompt, k):
    k = int(k)
    inputs = {"ppg": ppg, "keys": keys, "prompt": prompt}
    res = run_on_cores(inputs, k, trace=False)
    outs = res.results
    prompted = np.concatenate([m["out"] for m in outs], axis=0).reshape(B, 1, D)
    topk_cos = np.sum(
        np.stack([m["stats"][:, :8] for m in outs]).astype(np.float64)
    )
    ent = np.sum(np.stack([m["stats"][:, 8:] for m in outs]).astype(np.float64))
    score_sum = np.float32(k * B - topk_cos)
    entropy = np.float32(ent)
    return prompted, score_sum, entropy


# revision 42
# speedup vs baseline: 1.3158x; 1.0878x over previous
"""Trainium2 Bass kernel for topk_masking (L2Prompt-style prompted aggregation).

Computes, for ppg [B,1,D], keys/prompt [P,D], k:
  cos   = cosine_similarity(ppg[:,0,:], keys)          [B,P]
  score = 1 - cos; top-k smallest scores per row
  out   = (ppg + 0.5 * sum of the k selected prompt rows,
           sum of selected scores, entropy of softmax(score) summed over B,P)

Sharding: data-parallel over batch B across 8 NeuronCores; keys/prompt
replicated. Scalar partials reduced on host.

Per-core pipeline (BL=1024 rows, 8 b-tiles of 128):
  phase A   keys -> keysT = keys^T/||keys|| via PE transposes (DVE row-scale)
  s1a(bt)   load ppg tile, row norms, PE-transpose raw q -> qT
  s1b(bt)   cos matmul (contract D, f32), PSUM->SBUF scaled by 1/||q|| (DVE)
  stage2(bt) max8 top-k, exp/entropy partials, threshold mask (in-place),
            PE-transpose mask -> maskT, agg matmul maskT^T @ prompt (f32r),
            prompted = ppg + 0.5*agg (in-place), DMA out
  s1a/s1b run 2/1 b-tiles ahead so PE stays busy during the stats chain.

Self-contained: hardcodes shapes B=8192, P=1024, D=2048, 8 cores.
"""

import os
import sys

import numpy as np

for _p in ("/opt/trn_rl_repo",):
    if _p not in sys.path and os.path.isdir(_p):
        sys.path.insert(0, _p)

import concourse.mybir as mybir
from concourse import bacc as bacc_mod
from concourse import bass_utils
from concourse.masks import make_identity
from concourse.tile import TileContext

B, P, D = 8192, 1024, 2048
NCORES = 8
BL = B // NCORES          # rows per core = 1024
BT = BL // 128            # b-tiles per core = 8
PT = P // 128             # p-tiles = 8
DT = D // 128             # d-blocks = 16

F32 = mybir.dt.float32
F32R = mybir.dt.float32r
BF16 = mybir.dt.bfloat16

COS_F32R = os.environ.get("KERNEL_COS_F32R", "0") == "1"
AGG_F32R = os.environ.get("KERNEL_AGG_F32R", "1") == "1"


def build_kernel(k=5, cos_f32r=False, agg_f32r=True):
    """Build the per-core Bass program (identical on all 8 cores)."""
    assert 1 <= k <= 8
    nc = bacc_mod.Bacc(trn_type="TRN2")

    ppg_d = nc.dram_tensor("ppg", [BL, D], F32, kind="ExternalInput")
    keys_d = nc.dram_tensor("keys", [P, D], F32, kind="ExternalInput")
    prompt_d = nc.dram_tensor("prompt", [P, D], F32, kind="ExternalInput")
    out_d = nc.dram_tensor("out", [BL, D], F32, kind="ExternalOutput")
    # stats[:, j]   = sum of top-k cos for b-tile j's rows
    # stats[:, 8+j] = per-row entropy for b-tile j's rows
    stats_d = nc.dram_tensor("stats", [128, 16], F32, kind="ExternalOutput")

    cos_dt = F32R if cos_f32r else F32
    agg_dt = F32R if agg_f32r else F32

    with TileContext(nc) as tc:
        with (
            tc.tile_pool(name="singles", bufs=1) as singles,
            tc.tile_pool(name="rows", bufs=4) as rows,
            tc.tile_pool(name="big", bufs=2) as big,
            tc.tile_pool(name="mid", bufs=2) as mid,
            tc.tile_pool(name="small", bufs=3) as small,
            tc.tile_pool(name="scratch", bufs=1) as scratch,
            tc.tile_pool(name="psum_mm", bufs=4, space="PSUM") as psum_mm,
            tc.tile_pool(name="psum_tr", bufs=3, space="PSUM") as psum_tr,
        ):
            identity = singles.tile([128, 128], F32)
            make_identity(nc, identity)

            # prompt resident, natural [P, D] layout as [128, PT, D].
            # DMA'd per p-tile, emitted late (first needed by stage2(0)'s agg
            # matmul ~150us in) so it doesn't delay keys/ppg loads.
            prompt_sb = singles.tile([128, PT, D], agg_dt)
            prompt_src = prompt_d[:, :].rearrange("(n p) d -> p n d", p=128)
            if agg_f32r:
                prompt_src = prompt_src.bitcast(F32R)

            def load_prompt(pblk):
                nc.sync.dma_start(
                    prompt_sb[:, pblk, :], prompt_src[:, pblk, :]
                )

            # keysT resident: [128 (d within block), DT, P] = keys^T / kn
            keysT = singles.tile([128, DT, P], cos_dt)

            stats_sb = singles.tile([128, 16], F32)
            s_all = singles.tile([128, BT], F32)   # sum exp(-cos) per row
            u_all = singles.tile([128, BT], F32)   # sum cos*exp(-cos) per row

            # per-bt state carried between pipeline stages
            qnr_tiles = [None] * BT
            qT_tiles = [None] * BT
            cos_tiles = [None] * BT

            def phase_a_chunk(chunk):
                """Two keys p-tiles -> normalized keysT columns."""
                khat = []
                for j in range(2):
                    pt = chunk * 2 + j
                    ktile = rows.tile([128, D], F32, tag="rowtile")
                    nc.sync.dma_start(ktile, keys_d[pt * 128 : (pt + 1) * 128, :])
                    sq = scratch.tile([128, D], BF16, tag="sq")
                    kn2 = small.tile([128, 1], F32, tag="norm2")
                    nc.scalar.activation(
                        out=sq, in_=ktile,
                        func=mybir.ActivationFunctionType.Square,
                        accum_out=kn2,
                    )
                    kn = small.tile([128, 1], F32, tag="norm")
                    nc.scalar.sqrt(kn, kn2)
                    knr = small.tile([128, 1], F32, tag="nrecip")
                    nc.vector.reciprocal(knr, kn)
                    khat_t = rows.tile([128, D], F32, tag="rowtile")
                    nc.vector.tensor_scalar(
                        khat_t, ktile, knr, None, op0=mybir.AluOpType.mult
                    )
                    khat.append(khat_t)
                for dblk in range(DT):
                    pst = psum_tr.tile([128, 256], F32, tag="tr")
                    for j in range(2):
                        nc.tensor.transpose(
                            pst[:, j * 128 : (j + 1) * 128],
                            khat[j][:, dblk * 128 : (dblk + 1) * 128],
                            identity,
                        )
                    nc.vector.tensor_copy(
                        keysT[:, dblk, chunk * 256 : (chunk + 1) * 256], pst
                    )

            def s1a(bt):
                """Load ppg tile, row norms, transpose raw q -> qT."""
                ppg_t = rows.tile([128, D], F32, tag="rowtile")
                nc.sync.dma_start(ppg_t, ppg_d[bt * 128 : (bt + 1) * 128, :])
                sq = scratch.tile([128, D], BF16, tag="sq")
                qn2 = small.tile([128, 1], F32, tag="norm2")
                nc.scalar.activation(
                    out=sq, in_=ppg_t,
                    func=mybir.ActivationFunctionType.Square,
                    accum_out=qn2,
                )
                qn = small.tile([128, 1], F32, tag="norm")
                nc.scalar.sqrt(qn, qn2)
                qnr = small.tile([128, 1], F32, tag="nrecip")
                nc.vector.reciprocal(qnr, qn)

                qT = big.tile([128, DT, 128], cos_dt, tag="qT")
                for dgrp in range(DT // 4):
                    pst = psum_tr.tile([128, 512], F32, tag="tr")
                    for j in range(4):
                        dblk = dgrp * 4 + j
                        nc.tensor.transpose(
                            pst[:, j * 128 : (j + 1) * 128],
                            ppg_t[:, dblk * 128 : (dblk + 1) * 128],
                            identity,
                        )
                    nc.vector.tensor_copy(qT[:, dgrp * 4 : (dgrp + 1) * 4, :], pst)
                qnr_tiles[bt], qT_tiles[bt] = qnr, qT

            def s1b(bt):
                """cos matmul + PSUM->SBUF row-scaled copy (DVE)."""
                qT, qnr = qT_tiles[bt], qnr_tiles[bt]
                cos_sb = mid.tile([128, P], F32, tag="cos")
                for half in range(2):
                    psc = psum_mm.tile([128, 512], F32, tag="mm")
                    for kt in range(DT):
                        nc.tensor.matmul(
                            psc,
                            qT[:, kt, :],
                            keysT[:, kt, half * 512 : (half + 1) * 512],
                            start=(kt == 0),
                            stop=(kt == DT - 1),
                        )
                    nc.vector.tensor_scalar(
                        cos_sb[:, half * 512 : (half + 1) * 512],
                        psc, qnr, None, op0=mybir.AluOpType.mult,
                    )
                cos_tiles[bt] = cos_sb

            def stage2(bt):
                """top-k stats, entropy partials, mask, agg, output."""
                cos_sb = cos_tiles[bt]
                # re-load ppg for the final add (cheaper than pinning the
                # s1a tile across the whole pipeline: DMA has headroom)
                ppg_t = rows.tile([128, D], F32, tag="rowtile")
                nc.sync.dma_start(ppg_t, ppg_d[bt * 128 : (bt + 1) * 128, :])
                v8 = small.tile([128, 8], F32, tag="v8")
                nc.vector.max(out=v8, in_=cos_sb)
                nc.vector.tensor_reduce(
                    stats_sb[:, bt : bt + 1], v8[:, 0:k],
                    axis=mybir.AxisListType.X, op=mybir.AluOpType.add,
                )

                # p_i = exp(-cos_i); s = sum_i p_i   (|cos|<=1: no max-shift)
                p_t = mid.tile([128, P], F32, tag="p")
                nc.scalar.activation(
                    out=p_t, in_=cos_sb,
                    func=mybir.ActivationFunctionType.Exp,
                    scale=-1.0, accum_out=s_all[:, bt : bt + 1],
                )
                # u2 = sum_i p_i * cos_i  (H_row = log s + u2/s)
                # (shares the 4KB/partition "sq" scratch slot; value unused)
                tt_out = scratch.tile([128, P], F32, tag="sq")
                nc.vector.scalar_tensor_tensor(
                    out=tt_out, in0=p_t, scalar=1.0, in1=cos_sb,
                    op0=mybir.AluOpType.mult, op1=mybir.AluOpType.mult,
                    accum_out=u_all[:, bt : bt + 1],
                )

                # mask = cos >= kth largest, in place over cos_sb (all other
                # readers of cos are sequenced before this write)
                thr = v8[:, k - 1 : k]
                nc.vector.tensor_scalar(
                    cos_sb, cos_sb, thr, None, op0=mybir.AluOpType.is_ge
                )

                # maskT: [128 (p in block), PT, 128 (b)]
                maskT = mid.tile([128, PT, 128], agg_dt, tag="maskT")
                for pgrp in range(PT // 4):
                    pst = psum_tr.tile([128, 512], F32, tag="tr")
                    for j in range(4):
                        pblk = pgrp * 4 + j
                        nc.tensor.transpose(
                            pst[:, j * 128 : (j + 1) * 128],
                            cos_sb[:, pblk * 128 : (pblk + 1) * 128],
                            identity,
                        )
                    nc.vector.tensor_copy(
                        maskT[:, pgrp * 4 : (pgrp + 1) * 4, :], pst
                    )

                # agg = maskT^T @ prompt; prompted = ppg + 0.5*agg (in place)
                for nd in range(D // 512):
                    psa = psum_mm.tile([128, 512], F32, tag="mm")
                    for pblk in range(PT):
                        nc.tensor.matmul(
                            psa,
                            maskT[:, pblk, :],
                            prompt_sb[:, pblk, nd * 512 : (nd + 1) * 512],
                            start=(pblk == 0),
                            stop=(pblk == PT - 1),
                        )
                    nc.vector.scalar_tensor_tensor(
                        out=ppg_t[:, nd * 512 : (nd + 1) * 512],
                        in0=psa, scalar=0.5,
                        in1=ppg_t[:, nd * 512 : (nd + 1) * 512],
                        op0=mybir.AluOpType.mult, op1=mybir.AluOpType.add,
                    )
                nc.sync.dma_start(out_d[bt * 128 : (bt + 1) * 128, :], ppg_t)

            # ---- emission: phase A interleaved with early s1a, then the
            # ---- software-pipelined b-tile loop (s1a 2 ahead, s1b 1 ahead)
            for chunk in range(PT // 2):
                phase_a_chunk(chunk)
                if chunk < 2:
                    s1a(chunk)  # bt 0,1 early — fills phase-A PE gaps
                load_prompt(2 * chunk)
                load_prompt(2 * chunk + 1)
            s1b(0)
            for bt in range(BT):
                if bt + 2 < BT:
                    s1a(bt + 2)
                if bt + 1 < BT:
                    s1b(bt + 1)
                stage2(bt)

            # batched entropy epilogue: H_row = log s + u2/s
            sinv = small.tile([128, BT], F32, tag="sinv8")
            nc.vector.reciprocal(sinv, s_all)
            logs = small.tile([128, BT], F32, tag="logs8")
            nc.scalar.activation(
                out=logs, in_=s_all, func=mybir.ActivationFunctionType.Ln
            )
            usv = small.tile([128, BT], F32, tag="usv8")
            nc.vector.tensor_mul(usv, u_all, sinv)
            nc.vector.tensor_add(stats_sb[:, 8:16], logs, usv)

            nc.sync.dma_start(stats_d[:, :], stats_sb)

    nc.finalize()
    return nc


_CACHE = {}


def _get_nc(k):
    key = (int(k), COS_F32R, AGG_F32R)
    if key not in _CACHE:
        _CACHE[key] = build_kernel(k=key[0], cos_f32r=key[1], agg_f32r=key[2])
    return _CACHE[key]


_LAST_RESULTS = {}


def run_on_cores(inputs, k, trace=False):
    """Run the SPMD kernel on 8 cores; returns BassKernelResults."""
    nc = _get_nc(k)
    ppg = np.ascontiguousarray(
        np.asarray(inputs["ppg"], dtype=np.float32).reshape(B, D)
    )
    keys = np.ascontiguousarray(np.asarray(inputs["keys"], dtype=np.float32))
    prompt = np.ascontiguousarray(np.asarray(inputs["prompt"], dtype=np.float32))
    in_maps = [
        {
            "ppg": ppg[c * BL : (c + 1) * BL],
            "keys": keys,
            "prompt": prompt,
        }
        for c in range(NCORES)
    ]
    res = bass_utils.run_bass_kernel_spmd(
        nc, in_maps, core_ids=list(range(NCORES)), trace=trace
    )
    _LAST_RESULTS["res"] = res
    return res


def kernel(ppg, keys, prompt, k):
    k = int(k)
    inputs = {"ppg": ppg, "keys": keys, "prompt": prompt}
    res = run_on_cores(inputs, k, trace=False)
    outs = res.results
    prompted = np.concatenate([m["out"] for m in outs], axis=0).reshape(B, 1, D)
    topk_cos = np.sum(
        np.stack([m["stats"][:, :8] for m in outs]).astype(np.float64)
    )
    ent = np.sum(np.stack([m["stats"][:, 8:] for m in outs]).astype(np.float64))
    score_sum = np.float32(k * B - topk_cos)
    entropy = np.float32(ent)
    return prompted, score_sum, entropy


# revision 43
# speedup vs baseline: 1.4318x; 1.0881x over previous
"""Trainium2 Bass kernel for topk_masking (L2Prompt-style prompted aggregation).

Computes, for ppg [B,1,D], keys/prompt [P,D], k:
  cos   = cosine_similarity(ppg[:,0,:], keys)          [B,P]
  score = 1 - cos; top-k smallest scores per row
  out   = (ppg + 0.5 * sum of the k selected prompt rows,
           sum of selected scores, entropy of softmax(score) summed over B,P)

Sharding: data-parallel over batch B across 8 NeuronCores; keys/prompt
replicated. Scalar partials reduced on host.

Per-core pipeline (BL=1024 rows, 8 b-tiles of 128):
  phase A   keys -> keysT = keys^T/||keys|| via PE transposes (DVE row-scale)
  s1a(bt)   load ppg tile, row norms, PE-transpose raw q -> qT
  s1b(bt)   cos matmul (contract D, f32), PSUM->SBUF scaled by 1/||q|| (DVE)
  stage2(bt) max8 top-k, exp/entropy partials, threshold mask (in-place),
            PE-transpose mask -> maskT, agg matmul maskT^T @ prompt (f32r),
            prompted = ppg + 0.5*agg (in-place), DMA out
  s1a/s1b run 2/1 b-tiles ahead so PE stays busy during the stats chain.

Self-contained: hardcodes shapes B=8192, P=1024, D=2048, 8 cores.
"""

import os
import sys

import numpy as np

for _p in ("/opt/trn_rl_repo",):
    if _p not in sys.path and os.path.isdir(_p):
        sys.path.insert(0, _p)

import concourse.mybir as mybir
from concourse import bacc as bacc_mod
from concourse import bass_utils
from concourse.masks import make_identity
from concourse.tile import TileContext

B, P, D = 8192, 1024, 2048
NCORES = 8
BL = B // NCORES          # rows per core = 1024
BT = BL // 128            # b-tiles per core = 8
PT = P // 128             # p-tiles = 8
DT = D // 128             # d-blocks = 16

F32 = mybir.dt.float32
F32R = mybir.dt.float32r
BF16 = mybir.dt.bfloat16

COS_F32R = os.environ.get("KERNEL_COS_F32R", "0") == "1"
AGG_F32R = os.environ.get("KERNEL_AGG_F32R", "1") == "1"


def build_kernel(k=5, cos_f32r=False, agg_f32r=True):
    """Build the per-core Bass program (identical on all 8 cores)."""
    assert 1 <= k <= 8
    nc = bacc_mod.Bacc(trn_type="TRN2")

    ppg_d = nc.dram_tensor("ppg", [BL, D], F32, kind="ExternalInput")
    keys_d = nc.dram_tensor("keys", [P, D], F32, kind="ExternalInput")
    prompt_d = nc.dram_tensor("prompt", [P, D], F32, kind="ExternalInput")
    out_d = nc.dram_tensor("out", [BL, D], F32, kind="ExternalOutput")
    # stats[:, j]   = sum of top-k cos for b-tile j's rows
    # stats[:, 8+j] = per-row entropy for b-tile j's rows
    stats_d = nc.dram_tensor("stats", [128, 16], F32, kind="ExternalOutput")

    cos_dt = F32R if cos_f32r else F32
    agg_dt = F32R if agg_f32r else F32

    with TileContext(nc) as tc:
        with (
            tc.tile_pool(name="singles", bufs=1) as singles,
            tc.tile_pool(name="rows", bufs=4) as rows,
            tc.tile_pool(name="big", bufs=2) as big,
            tc.tile_pool(name="mid", bufs=2) as mid,
            tc.tile_pool(name="small", bufs=3) as small,
            tc.tile_pool(name="scratch", bufs=3) as scratch,
            tc.tile_pool(name="psum_mm", bufs=4, space="PSUM") as psum_mm,
            tc.tile_pool(name="psum_tr", bufs=3, space="PSUM") as psum_tr,
        ):
            identity = singles.tile([128, 128], F32)
            make_identity(nc, identity)

            # prompt resident, natural [P, D] layout as [128, PT, D].
            # DMA'd per p-tile, emitted late (first needed by stage2(0)'s agg
            # matmul ~150us in) so it doesn't delay keys/ppg loads.
            prompt_sb = singles.tile([128, PT, D], agg_dt)
            prompt_src = prompt_d[:, :].rearrange("(n p) d -> p n d", p=128)
            if agg_f32r:
                prompt_src = prompt_src.bitcast(F32R)

            def load_prompt(pblk):
                nc.sync.dma_start(
                    prompt_sb[:, pblk, :], prompt_src[:, pblk, :]
                )

            # keysT resident: [128 (d within block), DT, P] = keys^T / kn,
            # split into bf16 hi + lo so the cos matmul can run as
            # hi*hi + hi*lo + lo*hi at bf16 speed with ~fp32 accuracy
            keysT_hi = singles.tile([128, DT, P], BF16)
            keysT_lo = singles.tile([128, DT, P], BF16)

            stats_sb = singles.tile([128, 16], F32)
            s_all = singles.tile([128, BT], F32)   # sum exp(-cos) per row
            u_all = singles.tile([128, BT], F32)   # sum cos*exp(-cos) per row

            # per-bt state carried between pipeline stages
            qnr_tiles = [None] * BT
            qT_tiles = [None] * BT
            cos_tiles = [None] * BT

            def phase_a_chunk(chunk):
                """Two keys p-tiles -> normalized keysT columns."""
                khat = []
                for j in range(2):
                    pt = chunk * 2 + j
                    ktile = rows.tile([128, D], F32, tag="rowtile")
                    nc.sync.dma_start(ktile, keys_d[pt * 128 : (pt + 1) * 128, :])
                    sq = scratch.tile([128, D], BF16, tag="sq")
                    kn2 = small.tile([128, 1], F32, tag="norm2")
                    nc.scalar.activation(
                        out=sq, in_=ktile,
                        func=mybir.ActivationFunctionType.Square,
                        accum_out=kn2,
                    )
                    kn = small.tile([128, 1], F32, tag="norm")
                    nc.scalar.sqrt(kn, kn2)
                    knr = small.tile([128, 1], F32, tag="nrecip")
                    nc.vector.reciprocal(knr, kn)
                    khat_t = rows.tile([128, D], F32, tag="rowtile")
                    nc.vector.tensor_scalar(
                        khat_t, ktile, knr, None, op0=mybir.AluOpType.mult
                    )
                    khat.append(khat_t)
                for dblk in range(DT):
                    pst = psum_tr.tile([128, 256], F32, tag="tr")
                    for j in range(2):
                        nc.tensor.transpose(
                            pst[:, j * 128 : (j + 1) * 128],
                            khat[j][:, dblk * 128 : (dblk + 1) * 128],
                            identity,
                        )
                    hi = keysT_hi[:, dblk, chunk * 256 : (chunk + 1) * 256]
                    nc.vector.tensor_copy(hi, pst)
                    nc.vector.tensor_sub(
                        keysT_lo[:, dblk, chunk * 256 : (chunk + 1) * 256],
                        pst, hi,
                    )

            def s1a(bt):
                """Load ppg tile, row norms, transpose raw q -> qT."""
                ppg_t = rows.tile([128, D], F32, tag="rowtile")
                nc.sync.dma_start(ppg_t, ppg_d[bt * 128 : (bt + 1) * 128, :])
                sq = scratch.tile([128, D], BF16, tag="sq")
                qn2 = small.tile([128, 1], F32, tag="norm2")
                nc.scalar.activation(
                    out=sq, in_=ppg_t,
                    func=mybir.ActivationFunctionType.Square,
                    accum_out=qn2,
                )
                qn = small.tile([128, 1], F32, tag="norm")
                nc.scalar.sqrt(qn, qn2)
                qnr = small.tile([128, 1], F32, tag="nrecip")
                nc.vector.reciprocal(qnr, qn)

                qT_hi = big.tile([128, DT, 128], BF16, tag="qT_hi")
                qT_lo = big.tile([128, DT, 128], BF16, tag="qT_lo")
                for dgrp in range(DT // 4):
                    pst = psum_tr.tile([128, 512], F32, tag="tr")
                    for j in range(4):
                        dblk = dgrp * 4 + j
                        nc.tensor.transpose(
                            pst[:, j * 128 : (j + 1) * 128],
                            ppg_t[:, dblk * 128 : (dblk + 1) * 128],
                            identity,
                        )
                    hi = qT_hi[:, dgrp * 4 : (dgrp + 1) * 4, :]
                    nc.vector.tensor_copy(hi, pst)
                    nc.vector.tensor_sub(
                        qT_lo[:, dgrp * 4 : (dgrp + 1) * 4, :], pst, hi
                    )
                qnr_tiles[bt], qT_tiles[bt] = qnr, (qT_hi, qT_lo)

            def s1b(bt):
                """cos matmul + PSUM->SBUF row-scaled copy (DVE)."""
                (qT_hi, qT_lo), qnr = qT_tiles[bt], qnr_tiles[bt]
                cos_sb = mid.tile([128, P], F32, tag="cos")
                passes = (
                    (qT_hi, keysT_hi), (qT_hi, keysT_lo), (qT_lo, keysT_hi),
                )
                for half in range(2):
                    psc = psum_mm.tile([128, 512], F32, tag="mm")
                    for pi, (lhs, rhs) in enumerate(passes):
                        for kt in range(DT):
                            nc.tensor.matmul(
                                psc,
                                lhs[:, kt, :],
                                rhs[:, kt, half * 512 : (half + 1) * 512],
                                start=(pi == 0 and kt == 0),
                                stop=(pi == 2 and kt == DT - 1),
                            )
                    nc.vector.tensor_scalar(
                        cos_sb[:, half * 512 : (half + 1) * 512],
                        psc, qnr, None, op0=mybir.AluOpType.mult,
                    )
                cos_tiles[bt] = cos_sb

            def stage2(bt):
                """top-k stats, entropy partials, mask, agg, output."""
                cos_sb = cos_tiles[bt]
                # re-load ppg for the final add (cheaper than pinning the
                # s1a tile across the whole pipeline: DMA has headroom)
                ppg_t = rows.tile([128, D], F32, tag="rowtile")
                nc.sync.dma_start(ppg_t, ppg_d[bt * 128 : (bt + 1) * 128, :])
                v8 = small.tile([128, 8], F32, tag="v8")
                nc.vector.max(out=v8, in_=cos_sb)
                nc.vector.tensor_reduce(
                    stats_sb[:, bt : bt + 1], v8[:, 0:k],
                    axis=mybir.AxisListType.X, op=mybir.AluOpType.add,
                )

                # p_i = exp(-cos_i); s = sum_i p_i   (|cos|<=1: no max-shift)
                p_t = scratch.tile([128, P], F32, tag="sq")
                nc.scalar.activation(
                    out=p_t, in_=cos_sb,
                    func=mybir.ActivationFunctionType.Exp,
                    scale=-1.0, accum_out=s_all[:, bt : bt + 1],
                )
                # u2 = sum_i p_i * cos_i  (H_row = log s + u2/s)
                # (shares the 4KB/partition "sq" scratch slot; value unused)
                tt_out = scratch.tile([128, P], F32, tag="sq")
                nc.vector.scalar_tensor_tensor(
                    out=tt_out, in0=p_t, scalar=1.0, in1=cos_sb,
                    op0=mybir.AluOpType.mult, op1=mybir.AluOpType.mult,
                    accum_out=u_all[:, bt : bt + 1],
                )

                # mask = cos >= kth largest, in place over cos_sb (all other
                # readers of cos are sequenced before this write)
                thr = v8[:, k - 1 : k]
                nc.vector.tensor_scalar(
                    cos_sb, cos_sb, thr, None, op0=mybir.AluOpType.is_ge
                )

                # maskT: [128 (p in block), PT, 128 (b)]
                maskT = mid.tile([128, PT, 128], agg_dt, tag="maskT")
                for pgrp in range(PT // 4):
                    pst = psum_tr.tile([128, 512], F32, tag="tr")
                    for j in range(4):
                        pblk = pgrp * 4 + j
                        nc.tensor.transpose(
                            pst[:, j * 128 : (j + 1) * 128],
                            cos_sb[:, pblk * 128 : (pblk + 1) * 128],
                            identity,
                        )
                    nc.vector.tensor_copy(
                        maskT[:, pgrp * 4 : (pgrp + 1) * 4, :], pst
                    )

                # agg = maskT^T @ prompt; prompted = ppg + 0.5*agg (in place)
                for nd in range(D // 512):
                    psa = psum_mm.tile([128, 512], F32, tag="mm")
                    for pblk in range(PT):
                        nc.tensor.matmul(
                            psa,
                            maskT[:, pblk, :],
                            prompt_sb[:, pblk, nd * 512 : (nd + 1) * 512],
                            start=(pblk == 0),
                            stop=(pblk == PT - 1),
                        )
                    nc.vector.scalar_tensor_tensor(
                        out=ppg_t[:, nd * 512 : (nd + 1) * 512],
                        in0=psa, scalar=0.5,
                        in1=ppg_t[:, nd * 512 : (nd + 1) * 512],
                        op0=mybir.AluOpType.mult, op1=mybir.AluOpType.add,
                    )
                nc.sync.dma_start(out_d[bt * 128 : (bt + 1) * 128, :], ppg_t)

            # ---- emission: phase A interleaved with early s1a, then the
            # ---- software-pipelined b-tile loop (s1a 2 ahead, s1b 1 ahead)
            for chunk in range(PT // 2):
                phase_a_chunk(chunk)
                if chunk < 2:
                    s1a(chunk)  # bt 0,1 early — fills phase-A PE gaps
                load_prompt(2 * chunk)
                load_prompt(2 * chunk + 1)
            s1b(0)
            for bt in range(BT):
                if bt + 2 < BT:
                    s1a(bt + 2)
                if bt + 1 < BT:
                    s1b(bt + 1)
                stage2(bt)

            # batched entropy epilogue: H_row = log s + u2/s
            sinv = small.tile([128, BT], F32, tag="sinv8")
            nc.vector.reciprocal(sinv, s_all)
            logs = small.tile([128, BT], F32, tag="logs8")
            nc.scalar.activation(
                out=logs, in_=s_all, func=mybir.ActivationFunctionType.Ln
            )
            usv = small.tile([128, BT], F32, tag="usv8")
            nc.vector.tensor_mul(usv, u_all, sinv)
            nc.vector.tensor_add(stats_sb[:, 8:16], logs, usv)

            nc.sync.dma_start(stats_d[:, :], stats_sb)

    nc.finalize()
    return nc


_CACHE = {}


def _get_nc(k):
    key = (int(k), COS_F32R, AGG_F32R)
    if key not in _CACHE:
        _CACHE[key] = build_kernel(k=key[0], cos_f32r=key[1], agg_f32r=key[2])
    return _CACHE[key]


_LAST_RESULTS = {}


def run_on_cores(inputs, k, trace=False):
    """Run the SPMD kernel on 8 cores; returns BassKernelResults."""
    nc = _get_nc(k)
    ppg = np.ascontiguousarray(
        np.asarray(inputs["ppg"], dtype=np.float32).reshape(B, D)
    )
    keys = np.ascontiguousarray(np.asarray(inputs["keys"], dtype=np.float32))
    prompt = np.ascontiguousarray(np.asarray(inputs["prompt"], dtype=np.float32))
    in_maps = [
        {
            "ppg": ppg[c * BL : (c + 1) * BL],
            "keys": keys,
            "prompt": prompt,
        }
        for c in range(NCORES)
    ]
    res = bass_utils.run_bass_kernel_spmd(
        nc, in_maps, core_ids=list(range(NCORES)), trace=trace
    )
    _LAST_RESULTS["res"] = res
    return res


def kernel(ppg, keys, prompt, k):
    k = int(k)
    inputs = {"ppg": ppg, "keys": keys, "prompt": prompt}
    res = run_on_cores(inputs, k, trace=False)
    outs = res.results
    prompted = np.concatenate([m["out"] for m in outs], axis=0).reshape(B, 1, D)
    topk_cos = np.sum(
        np.stack([m["stats"][:, :8] for m in outs]).astype(np.float64)
    )
    ent = np.sum(np.stack([m["stats"][:, 8:] for m in outs]).astype(np.float64))
    score_sum = np.float32(k * B - topk_cos)
    entropy = np.float32(ent)
    return prompted, score_sum, entropy
